# revision 7
# baseline (speedup 1.0000x reference)
"""VQ codebook lookup (ClusteringLayer) Trainium2 kernel.

Reference semantics:
    x   = inputs.squeeze(-1)                       # (B, D)
    cur = latent_vectors[index]                    # (B, V, D)
    sim = l2norm(cur, -1) @ l2norm(x, -1)          # (B, V) cosine sims
    best = argmax(sim, -1)                         # (B,)
    out = cur[b, best[b]]                          # (B, D) un-normalized rows

Key facts used:
  * Normalizing x is a positive per-row scale -> does not change argmax.
  * sim for row b depends only on t = index[b]; there are only T=16 tables,
    so the reference's (B, V, D) gather + normalize collapses to table-level
    matmuls.
  * bf16 matmul provably flips the argmax for this input distribution
    (min fp32 top-2 gap ~8e-5), so matmul and argmax stay fp32.
  * The DVE argmax ops (max8 / find_index8) only run in 1x mode
    (~1.07 ns/elem over V=1024), so wall time is dominated by the number
    of 128-row argmax chunks. The structure below minimizes them.

Sharding (SPMD: all 8 cores execute one instruction stream, so the chunk /
segment structure must be core-independent):
  * Each core owns two tables (window 0 and window 1): chunk0 = first 128
    rows of table0, chunk1 = first 128 rows of table1.
  * Tables with more than 128 rows put their excess (<= 64 rows) in chunk2:
    partitions 0:64 hold excess of the core's OWN table0 (rhs = window 0),
    partitions 64:128 hold excess of table1 (rhs = window 1). The host
    pairs over-full tables onto the same cores so this always works for
    the contest distribution; unused partitions get zero x columns.
  * Per chunk: fp32 matmuls into a [128, 1024] PSUM tile, per-row argmax
    via max8 + find_index8, indirect-DMA gather of the winning raw rows
    (constant window offset per partition range), write out.
Host scatters rows back into batch order. A generic fallback program
(ceil(slots/8) single-table chunks per core) handles any distribution the
fast structure cannot.
"""

import os
import sys

for _p in ("/opt/trn_rl_repo", "/root/.axon_site/_ro/trn_rl_repo"):
    if os.path.isdir(_p) and _p not in sys.path:
        sys.path.insert(0, _p)

import numpy as np

# Problem constants (hardcoded per contest contract).
T, V, D = 16, 1024, 128
B = 2048
N_CORES = 8
PCHUNK = 128        # rows per chunk (PSUM partition width)
HALFP = 64          # chunk2 segment width (partition split)
NHALF = 512         # matmul free-dim half (PSUM bank limit for fp32)
WARMUP = 5          # PE p-state warm-up matmuls
EPS = 1e-12

_PROGRAM_CACHE = {}


def _emit_chunk(nc, bass, sb, ps_sim, tabn_sb, xt_sb, tabr, out, k, segs):
    """Emit one 128-row chunk: matmuls per segment, argmax, gathers, out.

    segs: list of (window, p0, p1) partition segments of this chunk.
    """
    from concourse import mybir
    f32 = mybir.dt.float32
    u32 = mybir.dt.uint32

    sim_ps = ps_sim.tile([PCHUNK, V], f32, tag="sim")
    for (w, p0, p1) in segs:
        lhs = xt_sb[:, k * PCHUNK + p0:k * PCHUNK + p1]
        for h in range(V // NHALF):
            rhs = tabn_sb[:, w * V + h * NHALF:w * V + (h + 1) * NHALF]
            nc.tensor.matmul(out=sim_ps[p0:p1, h * NHALF:(h + 1) * NHALF],
                             lhsT=lhs, rhs=rhs, start=True, stop=True)
    m8 = sb.tile([PCHUNK, 8], f32, tag=f"m8_{k}")
    nc.vector.max(out=m8[:], in_=sim_ps[:])
    v8 = sb.tile([PCHUNK, 8], u32, tag=f"v8_{k}")
    nc.vector.max_index(out=v8[:], in_max=m8[:], in_values=sim_ps[:])
    sel = sb.tile([PCHUNK, D], f32, tag=f"sel_{k}")
    for (w, p0, p1) in segs:
        nc.gpsimd.indirect_dma_start(
            out=sel[p0:p1, :], out_offset=None,
            in_=tabr[:],
            in_offset=bass.IndirectOffsetOnAxis(ap=v8[p0:p1, 0:1], axis=0),
            element_offset=w * V * D)
    out_eng = nc.sync if k % 2 == 0 else nc.scalar
    out_eng.dma_start(out=out[k * PCHUNK:(k + 1) * PCHUNK, :], in_=sel[:])


def _build_program(chunk_segs, ntab):
    """Build the per-core Bass program.

    chunk_segs: per chunk, tuple of (window, p0, p1) segments.
    ntab: number of table windows.
    """
    from concourse import bacc, bass, mybir
    from concourse.tile import TileContext

    f32 = mybir.dt.float32
    nchunk = len(chunk_segs)

    nc = bacc.Bacc(None, target_bir_lowering=False, debug=False,
                   num_devices=N_CORES)
    xt = nc.declare_dram_parameter("xt", [D, nchunk * PCHUNK], f32,
                                   isOutput=False)
    tabtn = nc.declare_dram_parameter("tabtn", [D, ntab * V], f32,
                                      isOutput=False)
    tabr = nc.declare_dram_parameter("tabr", [ntab * V, D], f32,
                                     isOutput=False)
    out = nc.declare_dram_parameter("out", [nchunk * PCHUNK, D], f32,
                                    isOutput=True)

    with TileContext(nc) as tc:
        with tc.tile_pool(name="sb", bufs=1) as sb, \
             tc.tile_pool(name="ps_sim", bufs=min(nchunk, 3),
                          space="PSUM") as ps_sim, \
             tc.tile_pool(name="ps_warm", bufs=1, space="PSUM") as ps_warm:
            # ---- loads: operands of the first matmuls lead the queues ----
            tabn_sb = sb.tile([D, ntab * V], f32)
            xt_sb = sb.tile([D, nchunk * PCHUNK], f32)
            nc.sync.dma_start(out=xt_sb[:], in_=xt[:])
            for w in range(ntab):
                for h in range(V // NHALF):
                    c0 = w * V + h * NHALF
                    eng = nc.scalar if h == 0 else nc.sync
                    eng.dma_start(out=tabn_sb[:, c0:c0 + NHALF],
                                  in_=tabtn[:, c0:c0 + NHALF])

            # ---- PE warm-up during the load wait (p-state ramp) ----
            bf16 = mybir.dt.bfloat16
            ones_col_bf = nc.const_aps.tensor(1.0, (D, 1), bf16)
            ones_warm_bf = nc.const_aps.tensor(1.0, (D, NHALF), bf16)
            warm_ps = ps_warm.tile([1, NHALF], f32, tag="warm")
            for _ in range(WARMUP):
                nc.tensor.matmul(out=warm_ps[:], lhsT=ones_col_bf,
                                 rhs=ones_warm_bf, start=True, stop=True)

            for k, segs in enumerate(chunk_segs):
                _emit_chunk(nc, bass, sb, ps_sim, tabn_sb, xt_sb, tabr,
                            out, k, segs)
    nc.compile()
    return nc


def _get_program(chunk_segs, ntab):
    key = (chunk_segs, ntab)
    if key not in _PROGRAM_CACHE:
        _PROGRAM_CACHE[key] = _build_program(chunk_segs, ntab)
    return _PROGRAM_CACHE[key]


FAST_SEGS = (
    ((0, 0, PCHUNK),),
    ((1, 0, PCHUNK),),
    ((0, 0, HALFP), (1, HALFP, PCHUNK)),
)


def _plan_fast(idx):
    """Pair tables onto cores: chunk0/chunk1 = main 128 rows of the core's
    two tables, chunk2 = their excess rows (<=64 each). Returns None if
    infeasible (some table has more than 192 rows)."""
    tabs = []
    for t in range(T):
        rows = np.nonzero(idx == t)[0]
        if len(rows) > PCHUNK + HALFP:
            return None
        tabs.append((t, rows))
    if len(tabs) != 2 * N_CORES:
        return None
    # Sort by count descending; pair i-th with (2N-1-i)-th so over-full
    # tables spread across cores (each core gets at most 2 excesses, one
    # per window -- any pairing satisfies that since each table has one
    # excess slot and each core has one seg per window).
    tabs.sort(key=lambda s: -len(s[1]))
    assign = []
    for c in range(N_CORES):
        a = tabs[c]
        b = tabs[2 * N_CORES - 1 - c]
        assign.append((a, b))
    return assign


def _plan_generic(idx):
    """Any-distribution plan: single-table slots of <=128 rows packed onto
    cores, nchunk = ceil(slots / 8), chunk k reads window k."""
    slots = []
    for t in np.unique(idx):
        rows = np.nonzero(idx == t)[0]
        for s0 in range(0, len(rows), PCHUNK):
            slots.append((int(t), rows[s0:s0 + PCHUNK]))
    nchunk = max(1, -(-len(slots) // N_CORES))
    slots.sort(key=lambda s: -len(s[1]))
    loads = [0] * N_CORES
    assign = [[] for _ in range(N_CORES)]
    for t, rows in slots:
        c = min(range(N_CORES),
                key=lambda c: (len(assign[c]) >= nchunk, loads[c]))
        assign[c].append((t, rows))
        loads[c] += len(rows)
    return nchunk, assign


def _numpy_fallback(x, latent_vectors, idx):
    out = np.empty((B, D), dtype=np.float32)
    for t in np.unique(idx):
        rows = np.nonzero(idx == t)[0]
        tab = latent_vectors[t]
        invn = 1.0 / np.sqrt(np.maximum((tab * tab).sum(-1), EPS))
        sims = (x[rows] @ tab.T) * invn[None, :]
        best = np.argmax(sims, axis=-1)
        out[rows] = tab[best]
    return out


def kernel(inputs, latent_vectors, index, _trace=False, _tmpdir=None):
    from concourse import bass_utils

    x = np.asarray(inputs, dtype=np.float32).reshape(B, D)
    lv = np.ascontiguousarray(np.asarray(latent_vectors, dtype=np.float32))
    idx = np.asarray(index).astype(np.int64)

    if idx.min() < 0 or idx.max() >= T:
        return _numpy_fallback(x, lv, idx)

    invn = 1.0 / np.sqrt(np.maximum((lv * lv).sum(-1), EPS))  # (T, V)

    fast = _plan_fast(idx)
    in_maps = []
    row_map = []  # per core: list of (out_row_start, rows)
    if fast is not None:
        nc = _get_program(FAST_SEGS, 2)
        for c in range(N_CORES):
            (ta, rows_a), (tb, rows_b) = fast[c]
            xt = np.zeros((D, 3 * PCHUNK), dtype=np.float32)
            tabtn = np.empty((D, 2 * V), dtype=np.float32)
            tabr = np.empty((2 * V, D), dtype=np.float32)
            rmap = []
            for w, (t, rows) in enumerate(((ta, rows_a), (tb, rows_b))):
                main, exc = rows[:PCHUNK], rows[PCHUNK:]
                xt[:, w * PCHUNK:w * PCHUNK + len(main)] = x[main].T
                rmap.append((w * PCHUNK, main))
                if len(exc):
                    p0 = 2 * PCHUNK + w * HALFP
                    xt[:, p0:p0 + len(exc)] = x[exc].T
                    rmap.append((p0, exc))
                tabtn[:, w * V:(w + 1) * V] = (lv[t] * invn[t][:, None]).T
                tabr[w * V:(w + 1) * V, :] = lv[t]
            in_maps.append({"xt": xt, "tabtn": tabtn, "tabr": tabr})
            row_map.append(rmap)
    else:
        nchunk, assign = _plan_generic(idx)
        chunk_segs = tuple(((k, 0, PCHUNK),) for k in range(nchunk))
        nc = _get_program(chunk_segs, nchunk)
        for c in range(N_CORES):
            xt = np.zeros((D, nchunk * PCHUNK), dtype=np.float32)
            tabtn = np.zeros((D, nchunk * V), dtype=np.float32)
            tabr = np.zeros((nchunk * V, D), dtype=np.float32)
            rmap = []
            for k, (t, rows) in enumerate(assign[c]):
                xt[:, k * PCHUNK:k * PCHUNK + len(rows)] = x[rows].T
                tabtn[:, k * V:(k + 1) * V] = (lv[t] * invn[t][:, None]).T
                tabr[k * V:(k + 1) * V, :] = lv[t]
                rmap.append((k * PCHUNK, rows))
            in_maps.append({"xt": xt, "tabtn": tabtn, "tabr": tabr})
            row_map.append(rmap)

    kw = {}
    if _trace:
        kw.update(trace=True, tmpdir=_tmpdir)
    res = bass_utils.run_bass_kernel_spmd(nc, in_maps,
                                          list(range(N_CORES)), **kw)

    out = np.empty((B, D), dtype=np.float32)
    for c in range(N_CORES):
        dev_out = res.results[c]["out"]
        for (r0, rows) in row_map[c]:
            out[rows] = dev_out[r0:r0 + len(rows)]
    if _trace:
        return out, res
    return out


# revision 11
# speedup vs baseline: 1.0563x; 1.0563x over previous
"""VQ codebook lookup (ClusteringLayer) Trainium2 kernel.

Reference semantics:
    x   = inputs.squeeze(-1)                       # (B, D)
    cur = latent_vectors[index]                    # (B, V, D)
    sim = l2norm(cur, -1) @ l2norm(x, -1)          # (B, V) cosine sims
    best = argmax(sim, -1)                         # (B,)
    out = cur[b, best[b]]                          # (B, D) un-normalized rows

Key facts used:
  * Normalizing x is a positive per-row scale -> does not change argmax.
  * sim for row b depends only on t = index[b]; there are only T=16 tables,
    so the reference's (B, V, D) gather + normalize collapses to table-level
    matmuls.
  * bf16 matmul provably flips the argmax for this input distribution
    (min fp32 top-2 gap ~8e-5), so matmul and argmax stay fp32.
  * The DVE argmax ops (max8 / find_index8) only run in 1x mode
    (~1.07 ns/elem over V=1024), so wall time is dominated by the number
    of 128-row argmax chunks. The structure below minimizes them.

Sharding (SPMD: all 8 cores execute one instruction stream, so the chunk /
segment structure must be core-independent):
  * Each core owns two tables (window 0 and window 1): chunk0 = first 128
    rows of table0, chunk1 = first 128 rows of table1.
  * Tables with more than 128 rows put their excess (<= 64 rows) in chunk2:
    partitions 0:64 hold excess of the core's OWN table0 (rhs = window 0),
    partitions 64:128 hold excess of table1 (rhs = window 1). The host
    pairs over-full tables onto the same cores so this always works for
    the contest distribution; unused partitions get zero x columns.
  * Per chunk: fp32 matmuls into a [128, 1024] PSUM tile, per-row argmax
    via max8 + find_index8, indirect-DMA gather of the winning raw rows
    (constant window offset per partition range), write out.
Host scatters rows back into batch order. A generic fallback program
(ceil(slots/8) single-table chunks per core) handles any distribution the
fast structure cannot.
"""

import os
import sys

for _p in ("/opt/trn_rl_repo", "/root/.axon_site/_ro/trn_rl_repo"):
    if os.path.isdir(_p) and _p not in sys.path:
        sys.path.insert(0, _p)

import numpy as np

# Problem constants (hardcoded per contest contract).
T, V, D = 16, 1024, 128
B = 2048
N_CORES = 8
PCHUNK = 128        # rows per chunk (PSUM partition width)
HALFP = 64          # chunk2 segment width (partition split)
NHALF = 512         # matmul free-dim half (PSUM bank limit for fp32)
WARMUP = 5          # PE p-state warm-up matmuls
EPS = 1e-12

_PROGRAM_CACHE = {}


def _emit_chunk(nc, bass, sb, ps_sim, tabn_sb, xt_sb, vb_sb, tabr, out,
                k, segs):
    """Emit one 128-row chunk: matmuls per segment, argmax, gather, out.

    segs: list of (window, p0, p1) partition segments of this chunk.
    Multi-segment chunks gather with per-row u32 base offsets (vb_sb),
    since partition-offset indirect DMAs crash the hardware DGE.
    """
    from concourse import mybir
    f32 = mybir.dt.float32
    u32 = mybir.dt.uint32

    sim_ps = ps_sim.tile([PCHUNK, V], f32, tag="sim")
    for (w, p0, p1) in segs:
        lhs = xt_sb[:, k * PCHUNK + p0:k * PCHUNK + p1]
        for h in range(V // NHALF):
            rhs = tabn_sb[:, w * V + h * NHALF:w * V + (h + 1) * NHALF]
            nc.tensor.matmul(out=sim_ps[p0:p1, h * NHALF:(h + 1) * NHALF],
                             lhsT=lhs, rhs=rhs, start=True, stop=True)
    m8 = sb.tile([PCHUNK, 8], f32, tag=f"m8_{k}")
    nc.vector.max(out=m8[:], in_=sim_ps[:])
    v8 = sb.tile([PCHUNK, 8], u32, tag=f"v8_{k}")
    nc.vector.max_index(out=v8[:], in_max=m8[:], in_values=sim_ps[:])
    sel = sb.tile([PCHUNK, D], f32, tag=f"sel_{k}")
    if len(segs) == 1:
        nc.gpsimd.indirect_dma_start(
            out=sel[:], out_offset=None,
            in_=tabr[:],
            in_offset=bass.IndirectOffsetOnAxis(ap=v8[:, 0:1], axis=0),
            element_offset=segs[0][0] * V * D)
    else:
        goff = sb.tile([PCHUNK, 1], u32, tag=f"goff_{k}")
        nc.gpsimd.tensor_tensor(out=goff[:], in0=v8[:, 0:1],
                                in1=vb_sb[:], op=mybir.AluOpType.add)
        nc.gpsimd.indirect_dma_start(
            out=sel[:], out_offset=None,
            in_=tabr[:],
            in_offset=bass.IndirectOffsetOnAxis(ap=goff[:], axis=0))
    out_eng = nc.sync if k % 2 == 0 else nc.scalar
    out_eng.dma_start(out=out[k * PCHUNK:(k + 1) * PCHUNK, :], in_=sel[:])


def _build_program(chunk_segs, ntab):
    """Build the per-core Bass program.

    chunk_segs: per chunk, tuple of (window, p0, p1) segments.
    ntab: number of table windows.
    """
    from concourse import bacc, bass, mybir
    from concourse.tile import TileContext

    f32 = mybir.dt.float32
    nchunk = len(chunk_segs)

    nc = bacc.Bacc(None, target_bir_lowering=False, debug=False,
                   num_devices=N_CORES)
    u32 = mybir.dt.uint32
    xt = nc.declare_dram_parameter("xt", [D, nchunk * PCHUNK], f32,
                                   isOutput=False)
    tabtn = nc.declare_dram_parameter("tabtn", [D, ntab * V], f32,
                                      isOutput=False)
    tabr = nc.declare_dram_parameter("tabr", [ntab * V, D], f32,
                                     isOutput=False)
    vbase = nc.declare_dram_parameter("vbase", [PCHUNK, 1], u32,
                                      isOutput=False)
    out = nc.declare_dram_parameter("out", [nchunk * PCHUNK, D], f32,
                                    isOutput=True)

    with TileContext(nc) as tc:
        with tc.tile_pool(name="sb", bufs=1) as sb, \
             tc.tile_pool(name="ps_sim", bufs=min(nchunk, 3),
                          space="PSUM") as ps_sim, \
             tc.tile_pool(name="ps_warm", bufs=1, space="PSUM") as ps_warm:
            # ---- loads: operands of the first matmuls lead the queues ----
            tabn_sb = sb.tile([D, ntab * V], f32)
            xt_sb = sb.tile([D, nchunk * PCHUNK], f32)
            vb_sb = sb.tile([PCHUNK, 1], u32)
            nc.sync.dma_start(out=xt_sb[:], in_=xt[:])
            nc.scalar.dma_start(out=vb_sb[:], in_=vbase[:])
            for w in range(ntab):
                for h in range(V // NHALF):
                    c0 = w * V + h * NHALF
                    eng = nc.scalar if h == 0 else nc.sync
                    eng.dma_start(out=tabn_sb[:, c0:c0 + NHALF],
                                  in_=tabtn[:, c0:c0 + NHALF])

            # ---- PE warm-up during the load wait (p-state ramp) ----
            bf16 = mybir.dt.bfloat16
            ones_col_bf = nc.const_aps.tensor(1.0, (D, 1), bf16)
            ones_warm_bf = nc.const_aps.tensor(1.0, (D, NHALF), bf16)
            warm_ps = ps_warm.tile([1, NHALF], f32, tag="warm")
            for _ in range(WARMUP):
                nc.tensor.matmul(out=warm_ps[:], lhsT=ones_col_bf,
                                 rhs=ones_warm_bf, start=True, stop=True)

            for k, segs in enumerate(chunk_segs):
                _emit_chunk(nc, bass, sb, ps_sim, tabn_sb, xt_sb, vb_sb,
                            tabr, out, k, segs)
    nc.compile()
    return nc


def _get_program(chunk_segs, ntab):
    key = (chunk_segs, ntab)
    if key not in _PROGRAM_CACHE:
        _PROGRAM_CACHE[key] = _build_program(chunk_segs, ntab)
    return _PROGRAM_CACHE[key]


FAST_SEGS = (
    ((0, 0, PCHUNK),),
    ((1, 0, PCHUNK),),
    ((0, 0, HALFP), (1, HALFP, PCHUNK)),
)


def _plan_fast(idx):
    """Pair tables onto cores: chunk0/chunk1 = main 128 rows of the core's
    two tables, chunk2 = their excess rows (<=64 each). Returns None if
    infeasible (some table has more than 192 rows)."""
    tabs = []
    for t in range(T):
        rows = np.nonzero(idx == t)[0]
        if len(rows) > PCHUNK + HALFP:
            return None
        tabs.append((t, rows))
    if len(tabs) != 2 * N_CORES:
        return None
    # Sort by count descending; pair i-th with (2N-1-i)-th so over-full
    # tables spread across cores (each core gets at most 2 excesses, one
    # per window -- any pairing satisfies that since each table has one
    # excess slot and each core has one seg per window).
    tabs.sort(key=lambda s: -len(s[1]))
    assign = []
    for c in range(N_CORES):
        a = tabs[c]
        b = tabs[2 * N_CORES - 1 - c]
        assign.append((a, b))
    return assign


def _plan_generic(idx):
    """Any-distribution plan: single-table slots of <=128 rows packed onto
    cores, nchunk = ceil(slots / 8), chunk k reads window k."""
    slots = []
    for t in np.unique(idx):
        rows = np.nonzero(idx == t)[0]
        for s0 in range(0, len(rows), PCHUNK):
            slots.append((int(t), rows[s0:s0 + PCHUNK]))
    nchunk = max(1, -(-len(slots) // N_CORES))
    slots.sort(key=lambda s: -len(s[1]))
    loads = [0] * N_CORES
    assign = [[] for _ in range(N_CORES)]
    for t, rows in slots:
        c = min(range(N_CORES),
                key=lambda c: (len(assign[c]) >= nchunk, loads[c]))
        assign[c].append((t, rows))
        loads[c] += len(rows)
    return nchunk, assign


def _numpy_fallback(x, latent_vectors, idx):
    out = np.empty((B, D), dtype=np.float32)
    for t in np.unique(idx):
        rows = np.nonzero(idx == t)[0]
        tab = latent_vectors[t]
        invn = 1.0 / np.sqrt(np.maximum((tab * tab).sum(-1), EPS))
        sims = (x[rows] @ tab.T) * invn[None, :]
        best = np.argmax(sims, axis=-1)
        out[rows] = tab[best]
    return out


def kernel(inputs, latent_vectors, index, _trace=False, _tmpdir=None):
    from concourse import bass_utils

    x = np.asarray(inputs, dtype=np.float32).reshape(B, D)
    lv = np.ascontiguousarray(np.asarray(latent_vectors, dtype=np.float32))
    idx = np.asarray(index).astype(np.int64)

    if idx.min() < 0 or idx.max() >= T:
        return _numpy_fallback(x, lv, idx)

    invn = 1.0 / np.sqrt(np.maximum((lv * lv).sum(-1), EPS))  # (T, V)

    fast = _plan_fast(idx)
    in_maps = []
    row_map = []  # per core: list of (out_row_start, rows)
    if fast is not None:
        nc = _get_program(FAST_SEGS, 2)
        for c in range(N_CORES):
            (ta, rows_a), (tb, rows_b) = fast[c]
            xt = np.zeros((D, 3 * PCHUNK), dtype=np.float32)
            tabtn = np.empty((D, 2 * V), dtype=np.float32)
            tabr = np.empty((2 * V, D), dtype=np.float32)
            rmap = []
            for w, (t, rows) in enumerate(((ta, rows_a), (tb, rows_b))):
                main, exc = rows[:PCHUNK], rows[PCHUNK:]
                xt[:, w * PCHUNK:w * PCHUNK + len(main)] = x[main].T
                rmap.append((w * PCHUNK, main))
                if len(exc):
                    p0 = 2 * PCHUNK + w * HALFP
                    xt[:, p0:p0 + len(exc)] = x[exc].T
                    rmap.append((p0, exc))
                tabtn[:, w * V:(w + 1) * V] = (lv[t] * invn[t][:, None]).T
                tabr[w * V:(w + 1) * V, :] = lv[t]
            vb = np.zeros((PCHUNK, 1), dtype=np.uint32)
            vb[HALFP:, 0] = V
            in_maps.append({"xt": xt, "tabtn": tabtn, "tabr": tabr,
                            "vbase": vb})
            row_map.append(rmap)
    else:
        nchunk, assign = _plan_generic(idx)
        chunk_segs = tuple(((k, 0, PCHUNK),) for k in range(nchunk))
        nc = _get_program(chunk_segs, nchunk)
        for c in range(N_CORES):
            xt = np.zeros((D, nchunk * PCHUNK), dtype=np.float32)
            tabtn = np.zeros((D, nchunk * V), dtype=np.float32)
            tabr = np.zeros((nchunk * V, D), dtype=np.float32)
            rmap = []
            for k, (t, rows) in enumerate(assign[c]):
                xt[:, k * PCHUNK:k * PCHUNK + len(rows)] = x[rows].T
                tabtn[:, k * V:(k + 1) * V] = (lv[t] * invn[t][:, None]).T
                tabr[k * V:(k + 1) * V, :] = lv[t]
                rmap.append((k * PCHUNK, rows))
            in_maps.append({"xt": xt, "tabtn": tabtn, "tabr": tabr,
                            "vbase": np.zeros((PCHUNK, 1), dtype=np.uint32)})
            row_map.append(rmap)

    kw = {}
    if _trace:
        kw.update(trace=True, tmpdir=_tmpdir)
    res = bass_utils.run_bass_kernel_spmd(nc, in_maps,
                                          list(range(N_CORES)), **kw)

    out = np.empty((B, D), dtype=np.float32)
    for c in range(N_CORES):
        dev_out = res.results[c]["out"]
        for (r0, rows) in row_map[c]:
            out[rows] = dev_out[r0:r0 + len(rows)]
    if _trace:
        return out, res
    return out


# revision 13
# speedup vs baseline: 1.0758x; 1.0184x over previous
"""VQ codebook lookup (ClusteringLayer) Trainium2 kernel.

Reference semantics:
    x   = inputs.squeeze(-1)                       # (B, D)
    cur = latent_vectors[index]                    # (B, V, D)
    sim = l2norm(cur, -1) @ l2norm(x, -1)          # (B, V) cosine sims
    best = argmax(sim, -1)                         # (B,)
    out = cur[b, best[b]]                          # (B, D) un-normalized rows

Key facts used:
  * Normalizing x is a positive per-row scale -> does not change argmax.
  * sim for row b depends only on t = index[b]; there are only T=16 tables,
    so the reference's (B, V, D) gather + normalize collapses to table-level
    matmuls.
  * bf16 matmul provably flips the argmax for this input distribution
    (min fp32 top-2 gap ~8e-5), so matmul and argmax stay fp32.
  * The DVE argmax ops (max8 / find_index8) only run in 1x mode
    (~1.07 ns/elem over V=1024), so wall time is dominated by the number
    of 128-row argmax chunks. The structure below minimizes them.

Sharding (SPMD: all 8 cores execute one instruction stream, so the chunk /
segment structure must be core-independent):
  * Each core owns two tables (window 0 and window 1): chunk0 = first 128
    rows of table0, chunk1 = first 128 rows of table1.
  * Tables with more than 128 rows put their excess (<= 64 rows) in chunk2:
    partitions 0:64 hold excess of the core's OWN table0 (rhs = window 0),
    partitions 64:128 hold excess of table1 (rhs = window 1). The host
    pairs over-full tables onto the same cores so this always works for
    the contest distribution; unused partitions get zero x columns.
  * Per chunk: fp32 matmuls into a [128, 1024] PSUM tile, per-row argmax
    via max8 + find_index8, indirect-DMA gather of the winning raw rows
    (constant window offset per partition range), write out.
Host scatters rows back into batch order. A generic fallback program
(ceil(slots/8) single-table chunks per core) handles any distribution the
fast structure cannot.
"""

import os
import sys

for _p in ("/opt/trn_rl_repo", "/root/.axon_site/_ro/trn_rl_repo"):
    if os.path.isdir(_p) and _p not in sys.path:
        sys.path.insert(0, _p)

import numpy as np

# Problem constants (hardcoded per contest contract).
T, V, D = 16, 1024, 128
B = 2048
N_CORES = 8
PCHUNK = 128        # rows per chunk (PSUM partition width)
HALFP = 64          # chunk2 segment width (partition split)
NHALF = 512         # matmul free-dim half (PSUM bank limit for fp32)
WARMUP = 9          # PE p-state warm-up matmuls
EPS = 1e-12

_PROGRAM_CACHE = {}


def _emit_chunk(nc, bass, sb, ps_sim, tabn_sb, xt_sb, vb_sb, tabr, out,
                k, segs):
    """Emit one 128-row chunk: matmuls per segment, argmax, gather, out.

    segs: list of (window, p0, p1) partition segments of this chunk.
    Multi-segment chunks gather with per-row u32 base offsets (vb_sb),
    since partition-offset indirect DMAs crash the hardware DGE.
    """
    from concourse import mybir
    f32 = mybir.dt.float32
    u32 = mybir.dt.uint32

    sim_ps = ps_sim.tile([PCHUNK, V], f32, tag="sim")
    for (w, p0, p1) in segs:
        lhs = xt_sb[:, k * PCHUNK + p0:k * PCHUNK + p1]
        for h in range(V // NHALF):
            rhs = tabn_sb[w][h][:]
            nc.tensor.matmul(out=sim_ps[p0:p1, h * NHALF:(h + 1) * NHALF],
                             lhsT=lhs, rhs=rhs, start=True, stop=True)
    m8 = sb.tile([PCHUNK, 8], f32, tag=f"m8_{k}")
    nc.vector.max(out=m8[:], in_=sim_ps[:])
    v8 = sb.tile([PCHUNK, 8], u32, tag=f"v8_{k}")
    nc.vector.max_index(out=v8[:], in_max=m8[:], in_values=sim_ps[:])
    sel = sb.tile([PCHUNK, D], f32, tag=f"sel_{k}")
    if len(segs) == 1:
        nc.gpsimd.indirect_dma_start(
            out=sel[:], out_offset=None,
            in_=tabr[:],
            in_offset=bass.IndirectOffsetOnAxis(ap=v8[:, 0:1], axis=0),
            element_offset=segs[0][0] * V * D)
    else:
        goff = sb.tile([PCHUNK, 1], u32, tag=f"goff_{k}")
        nc.gpsimd.tensor_tensor(out=goff[:], in0=v8[:, 0:1],
                                in1=vb_sb[:], op=mybir.AluOpType.add)
        nc.gpsimd.indirect_dma_start(
            out=sel[:], out_offset=None,
            in_=tabr[:],
            in_offset=bass.IndirectOffsetOnAxis(ap=goff[:], axis=0))
    out_eng = nc.sync if k % 2 == 0 else nc.scalar
    out_eng.dma_start(out=out[k * PCHUNK:(k + 1) * PCHUNK, :], in_=sel[:])


def _build_program(chunk_segs, ntab):
    """Build the per-core Bass program.

    chunk_segs: per chunk, tuple of (window, p0, p1) segments.
    ntab: number of table windows.
    """
    from concourse import bacc, bass, mybir
    from concourse.tile import TileContext

    f32 = mybir.dt.float32
    nchunk = len(chunk_segs)

    nc = bacc.Bacc(None, target_bir_lowering=False, debug=False,
                   num_devices=N_CORES)
    u32 = mybir.dt.uint32
    xt = nc.declare_dram_parameter("xt", [D, nchunk * PCHUNK], f32,
                                   isOutput=False)
    tabtn = nc.declare_dram_parameter("tabtn", [D, ntab * V], f32,
                                      isOutput=False)
    tabr = nc.declare_dram_parameter("tabr", [ntab * V, D], f32,
                                     isOutput=False)
    vbase = nc.declare_dram_parameter("vbase", [PCHUNK, 1], u32,
                                      isOutput=False)
    out = nc.declare_dram_parameter("out", [nchunk * PCHUNK, D], f32,
                                    isOutput=True)

    with TileContext(nc) as tc:
        with tc.tile_pool(name="sb", bufs=1) as sb, \
             tc.tile_pool(name="ps_sim", bufs=min(nchunk, 3),
                          space="PSUM") as ps_sim, \
             tc.tile_pool(name="ps_warm", bufs=1, space="PSUM") as ps_warm:
            # ---- loads: operands of the first matmuls lead the queues.
            # One tile per (window, half) so each matmul waits only on its
            # own 256 KB slice, not the whole table load. ----
            tabn_sb = [[sb.tile([D, NHALF], f32, name=f"tn{w}_{h}")
                        for h in range(V // NHALF)] for w in range(ntab)]
            xt_sb = sb.tile([D, nchunk * PCHUNK], f32)
            vb_sb = sb.tile([PCHUNK, 1], u32)
            nc.sync.dma_start(out=xt_sb[:], in_=xt[:])
            nc.scalar.dma_start(out=vb_sb[:], in_=vbase[:])
            for w in range(ntab):
                for h in range(V // NHALF):
                    c0 = w * V + h * NHALF
                    eng = nc.scalar if h == 0 else nc.sync
                    eng.dma_start(out=tabn_sb[w][h][:],
                                  in_=tabtn[:, c0:c0 + NHALF])

            # ---- PE warm-up during the load wait (p-state ramp) ----
            bf16 = mybir.dt.bfloat16
            ones_col_bf = nc.const_aps.tensor(1.0, (D, 1), bf16)
            ones_warm_bf = nc.const_aps.tensor(1.0, (D, NHALF), bf16)
            warm_ps = ps_warm.tile([1, NHALF], f32, tag="warm")
            for _ in range(WARMUP):
                nc.tensor.matmul(out=warm_ps[:], lhsT=ones_col_bf,
                                 rhs=ones_warm_bf, start=True, stop=True)

            for k, segs in enumerate(chunk_segs):
                _emit_chunk(nc, bass, sb, ps_sim, tabn_sb, xt_sb, vb_sb,
                            tabr, out, k, segs)
    nc.compile()
    return nc


def _get_program(chunk_segs, ntab):
    key = (chunk_segs, ntab)
    if key not in _PROGRAM_CACHE:
        _PROGRAM_CACHE[key] = _build_program(chunk_segs, ntab)
    return _PROGRAM_CACHE[key]


FAST_SEGS = (
    ((0, 0, PCHUNK),),
    ((1, 0, PCHUNK),),
    ((0, 0, HALFP), (1, HALFP, PCHUNK)),
)


def _plan_fast(idx):
    """Pair tables onto cores: chunk0/chunk1 = main 128 rows of the core's
    two tables, chunk2 = their excess rows (<=64 each). Returns None if
    infeasible (some table has more than 192 rows)."""
    tabs = []
    for t in range(T):
        rows = np.nonzero(idx == t)[0]
        if len(rows) > PCHUNK + HALFP:
            return None
        tabs.append((t, rows))
    if len(tabs) != 2 * N_CORES:
        return None
    # Sort by count descending; pair i-th with (2N-1-i)-th so over-full
    # tables spread across cores (each core gets at most 2 excesses, one
    # per window -- any pairing satisfies that since each table has one
    # excess slot and each core has one seg per window).
    tabs.sort(key=lambda s: -len(s[1]))
    assign = []
    for c in range(N_CORES):
        a = tabs[c]
        b = tabs[2 * N_CORES - 1 - c]
        assign.append((a, b))
    return assign


def _plan_generic(idx):
    """Any-distribution plan: single-table slots of <=128 rows packed onto
    cores, nchunk = ceil(slots / 8), chunk k reads window k."""
    slots = []
    for t in np.unique(idx):
        rows = np.nonzero(idx == t)[0]
        for s0 in range(0, len(rows), PCHUNK):
            slots.append((int(t), rows[s0:s0 + PCHUNK]))
    nchunk = max(1, -(-len(slots) // N_CORES))
    slots.sort(key=lambda s: -len(s[1]))
    loads = [0] * N_CORES
    assign = [[] for _ in range(N_CORES)]
    for t, rows in slots:
        c = min(range(N_CORES),
                key=lambda c: (len(assign[c]) >= nchunk, loads[c]))
        assign[c].append((t, rows))
        loads[c] += len(rows)
    return nchunk, assign


def _numpy_fallback(x, latent_vectors, idx):
    out = np.empty((B, D), dtype=np.float32)
    for t in np.unique(idx):
        rows = np.nonzero(idx == t)[0]
        tab = latent_vectors[t]
        invn = 1.0 / np.sqrt(np.maximum((tab * tab).sum(-1), EPS))
        sims = (x[rows] @ tab.T) * invn[None, :]
        best = np.argmax(sims, axis=-1)
        out[rows] = tab[best]
    return out


def kernel(inputs, latent_vectors, index, _trace=False, _tmpdir=None):
    from concourse import bass_utils

    x = np.asarray(inputs, dtype=np.float32).reshape(B, D)
    lv = np.ascontiguousarray(np.asarray(latent_vectors, dtype=np.float32))
    idx = np.asarray(index).astype(np.int64)

    if idx.min() < 0 or idx.max() >= T:
        return _numpy_fallback(x, lv, idx)

    invn = 1.0 / np.sqrt(np.maximum((lv * lv).sum(-1), EPS))  # (T, V)

    fast = _plan_fast(idx)
    in_maps = []
    row_map = []  # per core: list of (out_row_start, rows)
    if fast is not None:
        nc = _get_program(FAST_SEGS, 2)
        for c in range(N_CORES):
            (ta, rows_a), (tb, rows_b) = fast[c]
            xt = np.zeros((D, 3 * PCHUNK), dtype=np.float32)
            tabtn = np.empty((D, 2 * V), dtype=np.float32)
            tabr = np.empty((2 * V, D), dtype=np.float32)
            rmap = []
            for w, (t, rows) in enumerate(((ta, rows_a), (tb, rows_b))):
                main, exc = rows[:PCHUNK], rows[PCHUNK:]
                xt[:, w * PCHUNK:w * PCHUNK + len(main)] = x[main].T
                rmap.append((w * PCHUNK, main))
                if len(exc):
                    p0 = 2 * PCHUNK + w * HALFP
                    xt[:, p0:p0 + len(exc)] = x[exc].T
                    rmap.append((p0, exc))
                tabtn[:, w * V:(w + 1) * V] = (lv[t] * invn[t][:, None]).T
                tabr[w * V:(w + 1) * V, :] = lv[t]
            vb = np.zeros((PCHUNK, 1), dtype=np.uint32)
            vb[HALFP:, 0] = V
            in_maps.append({"xt": xt, "tabtn": tabtn, "tabr": tabr,
                            "vbase": vb})
            row_map.append(rmap)
    else:
        nchunk, assign = _plan_generic(idx)
        chunk_segs = tuple(((k, 0, PCHUNK),) for k in range(nchunk))
        nc = _get_program(chunk_segs, nchunk)
        for c in range(N_CORES):
            xt = np.zeros((D, nchunk * PCHUNK), dtype=np.float32)
            tabtn = np.zeros((D, nchunk * V), dtype=np.float32)
            tabr = np.zeros((nchunk * V, D), dtype=np.float32)
            rmap = []
            for k, (t, rows) in enumerate(assign[c]):
                xt[:, k * PCHUNK:k * PCHUNK + len(rows)] = x[rows].T
                tabtn[:, k * V:(k + 1) * V] = (lv[t] * invn[t][:, None]).T
                tabr[k * V:(k + 1) * V, :] = lv[t]
                rmap.append((k * PCHUNK, rows))
            in_maps.append({"xt": xt, "tabtn": tabtn, "tabr": tabr,
                            "vbase": np.zeros((PCHUNK, 1), dtype=np.uint32)})
            row_map.append(rmap)

    kw = {}
    if _trace:
        kw.update(trace=True, tmpdir=_tmpdir)
    res = bass_utils.run_bass_kernel_spmd(nc, in_maps,
                                          list(range(N_CORES)), **kw)

    out = np.empty((B, D), dtype=np.float32)
    for c in range(N_CORES):
        dev_out = res.results[c]["out"]
        for (r0, rows) in row_map[c]:
            out[rows] = dev_out[r0:r0 + len(rows)]
    if _trace:
        return out, res
    return out


# revision 15
# speedup vs baseline: 1.1182x; 1.0395x over previous
"""VQ codebook lookup (ClusteringLayer) Trainium2 kernel.

Reference semantics:
    x   = inputs.squeeze(-1)                       # (B, D)
    cur = latent_vectors[index]                    # (B, V, D)
    sim = l2norm(cur, -1) @ l2norm(x, -1)          # (B, V) cosine sims
    best = argmax(sim, -1)                         # (B,)
    out = cur[b, best[b]]                          # (B, D) un-normalized rows

Key facts used:
  * Normalizing x is a positive per-row scale -> does not change argmax.
  * sim for row b depends only on t = index[b]; there are only T=16 tables,
    so the reference's (B, V, D) gather + normalize collapses to table-level
    matmuls.
  * bf16 matmul provably flips the argmax for this input distribution
    (min fp32 top-2 gap ~8e-5), so matmul and argmax stay fp32.
  * The DVE argmax ops (max8 / find_index8) only run in 1x mode
    (~1.07 ns/elem over V=1024), so wall time is dominated by the number
    of 128-row argmax chunks. The structure below minimizes them.

Sharding (SPMD: all 8 cores execute one instruction stream, so the chunk /
segment structure must be core-independent):
  * Each core owns two tables (window 0 and window 1): chunk0 = first 128
    rows of table0, chunk1 = first 128 rows of table1.
  * Tables with more than 128 rows put their excess (<= 64 rows) in chunk2:
    partitions 0:64 hold excess of the core's OWN table0 (rhs = window 0),
    partitions 64:128 hold excess of table1 (rhs = window 1). The host
    pairs over-full tables onto the same cores so this always works for
    the contest distribution; unused partitions get zero x columns.
  * Per chunk: fp32 matmuls into a [128, 1024] PSUM tile, per-row argmax
    via max8 + find_index8, indirect-DMA gather of the winning raw rows
    (constant window offset per partition range), write out.
Host scatters rows back into batch order. A generic fallback program
(ceil(slots/8) single-table chunks per core) handles any distribution the
fast structure cannot.
"""

import os
import sys

for _p in ("/opt/trn_rl_repo", "/root/.axon_site/_ro/trn_rl_repo"):
    if os.path.isdir(_p) and _p not in sys.path:
        sys.path.insert(0, _p)

import numpy as np

# Problem constants (hardcoded per contest contract).
T, V, D = 16, 1024, 128
B = 2048
N_CORES = 8
PCHUNK = 128        # rows per chunk (PSUM partition width)
HALFP = 64          # chunk2 segment width (partition split)
NHALF = 512         # matmul free-dim half (PSUM bank limit for fp32)
WARMUP = 16          # PE p-state warm-up matmuls
EPS = 1e-12

_PROGRAM_CACHE = {}


def _emit_chunk(nc, bass, sb, ps_sim, tabn_sb, xt_sb, vb_sb, tabr, out,
                k, segs):
    """Emit one 128-row chunk: matmuls per segment, argmax, gather, out.

    segs: list of (window, p0, p1) partition segments of this chunk.
    Multi-segment chunks gather with per-row u32 base offsets (vb_sb),
    since partition-offset indirect DMAs crash the hardware DGE.
    """
    from concourse import mybir
    f32 = mybir.dt.float32
    u32 = mybir.dt.uint32

    sim_ps = ps_sim.tile([PCHUNK, V], f32, tag="sim")
    for (w, p0, p1) in segs:
        lhs = xt_sb[:, k * PCHUNK + p0:k * PCHUNK + p1]
        for h in range(V // NHALF):
            rhs = tabn_sb[w][h][:]
            nc.tensor.matmul(out=sim_ps[p0:p1, h * NHALF:(h + 1) * NHALF],
                             lhsT=lhs, rhs=rhs, start=True, stop=True)
    m8 = sb.tile([PCHUNK, 8], f32, tag=f"m8_{k}")
    nc.vector.max(out=m8[:], in_=sim_ps[:])
    v8 = sb.tile([PCHUNK, 8], u32, tag=f"v8_{k}")
    nc.vector.max_index(out=v8[:], in_max=m8[:], in_values=sim_ps[:])
    sel = sb.tile([PCHUNK, D], f32, tag=f"sel_{k}")
    if len(segs) == 1:
        nc.gpsimd.indirect_dma_start(
            out=sel[:], out_offset=None,
            in_=tabr[:],
            in_offset=bass.IndirectOffsetOnAxis(ap=v8[:, 0:1], axis=0),
            element_offset=segs[0][0] * V * D)
    else:
        goff = sb.tile([PCHUNK, 1], u32, tag=f"goff_{k}")
        nc.gpsimd.tensor_tensor(out=goff[:], in0=v8[:, 0:1],
                                in1=vb_sb[:], op=mybir.AluOpType.add)
        nc.gpsimd.indirect_dma_start(
            out=sel[:], out_offset=None,
            in_=tabr[:],
            in_offset=bass.IndirectOffsetOnAxis(ap=goff[:], axis=0))
    out_eng = nc.sync if k % 2 == 0 else nc.scalar
    out_eng.dma_start(out=out[k * PCHUNK:(k + 1) * PCHUNK, :], in_=sel[:])


def _build_program(chunk_segs, ntab):
    """Build the per-core Bass program.

    chunk_segs: per chunk, tuple of (window, p0, p1) segments.
    ntab: number of table windows.
    """
    from concourse import bacc, bass, mybir
    from concourse.tile import TileContext

    f32 = mybir.dt.float32
    nchunk = len(chunk_segs)

    nc = bacc.Bacc(None, target_bir_lowering=False, debug=False,
                   num_devices=N_CORES)
    u32 = mybir.dt.uint32
    xt = nc.declare_dram_parameter("xt", [D, nchunk * PCHUNK], f32,
                                   isOutput=False)
    tabtn = nc.declare_dram_parameter("tabtn", [D, ntab * V], f32,
                                      isOutput=False)
    tabr = nc.declare_dram_parameter("tabr", [ntab * V, D], f32,
                                     isOutput=False)
    vbase = nc.declare_dram_parameter("vbase", [PCHUNK, 1], u32,
                                      isOutput=False)
    out = nc.declare_dram_parameter("out", [nchunk * PCHUNK, D], f32,
                                    isOutput=True)

    with TileContext(nc) as tc:
        with tc.tile_pool(name="sb", bufs=1) as sb, \
             tc.tile_pool(name="ps_sim", bufs=min(nchunk, 3),
                          space="PSUM") as ps_sim, \
             tc.tile_pool(name="ps_warm", bufs=1, space="PSUM") as ps_warm:
            # ---- loads: operands of the first matmuls lead the queues.
            # One tile per (window, half) so each matmul waits only on its
            # own 256 KB slice, not the whole table load. ----
            tabn_sb = [[sb.tile([D, NHALF], f32, name=f"tn{w}_{h}")
                        for h in range(V // NHALF)] for w in range(ntab)]
            xt_sb = sb.tile([D, nchunk * PCHUNK], f32)
            vb_sb = sb.tile([PCHUNK, 1], u32)
            nc.sync.dma_start(out=xt_sb[:], in_=xt[:])
            nc.scalar.dma_start(out=vb_sb[:], in_=vbase[:])
            # Four parallel HWDGE queues; window 0 (first consumed) leads.
            engs = [nc.scalar, nc.gpsimd, nc.sync]
            qi = 0
            for w in range(ntab):
                for h in range(V // NHALF):
                    c0 = w * V + h * NHALF
                    engs[qi % 3].dma_start(out=tabn_sb[w][h][:],
                                           in_=tabtn[:, c0:c0 + NHALF])
                    qi += 1

            # ---- PE warm-up during the load wait (p-state ramp) ----
            bf16 = mybir.dt.bfloat16
            ones_col_bf = nc.const_aps.tensor(1.0, (D, 1), bf16)
            ones_warm_bf = nc.const_aps.tensor(1.0, (D, NHALF), bf16)
            warm_ps = ps_warm.tile([1, NHALF], f32, tag="warm")
            for _ in range(WARMUP):
                nc.tensor.matmul(out=warm_ps[:], lhsT=ones_col_bf,
                                 rhs=ones_warm_bf, start=True, stop=True)

            for k, segs in enumerate(chunk_segs):
                _emit_chunk(nc, bass, sb, ps_sim, tabn_sb, xt_sb, vb_sb,
                            tabr, out, k, segs)
    nc.compile()
    return nc


def _get_program(chunk_segs, ntab):
    key = (chunk_segs, ntab)
    if key not in _PROGRAM_CACHE:
        _PROGRAM_CACHE[key] = _build_program(chunk_segs, ntab)
    return _PROGRAM_CACHE[key]


FAST_SEGS = (
    ((0, 0, PCHUNK),),
    ((1, 0, PCHUNK),),
    ((0, 0, HALFP), (1, HALFP, PCHUNK)),
)


def _plan_fast(idx):
    """Pair tables onto cores: chunk0/chunk1 = main 128 rows of the core's
    two tables, chunk2 = their excess rows (<=64 each). Returns None if
    infeasible (some table has more than 192 rows)."""
    tabs = []
    for t in range(T):
        rows = np.nonzero(idx == t)[0]
        if len(rows) > PCHUNK + HALFP:
            return None
        tabs.append((t, rows))
    if len(tabs) != 2 * N_CORES:
        return None
    # Sort by count descending; pair i-th with (2N-1-i)-th so over-full
    # tables spread across cores (each core gets at most 2 excesses, one
    # per window -- any pairing satisfies that since each table has one
    # excess slot and each core has one seg per window).
    tabs.sort(key=lambda s: -len(s[1]))
    assign = []
    for c in range(N_CORES):
        a = tabs[c]
        b = tabs[2 * N_CORES - 1 - c]
        assign.append((a, b))
    return assign


def _plan_generic(idx):
    """Any-distribution plan: single-table slots of <=128 rows packed onto
    cores, nchunk = ceil(slots / 8), chunk k reads window k."""
    slots = []
    for t in np.unique(idx):
        rows = np.nonzero(idx == t)[0]
        for s0 in range(0, len(rows), PCHUNK):
            slots.append((int(t), rows[s0:s0 + PCHUNK]))
    nchunk = max(1, -(-len(slots) // N_CORES))
    slots.sort(key=lambda s: -len(s[1]))
    loads = [0] * N_CORES
    assign = [[] for _ in range(N_CORES)]
    for t, rows in slots:
        c = min(range(N_CORES),
                key=lambda c: (len(assign[c]) >= nchunk, loads[c]))
        assign[c].append((t, rows))
        loads[c] += len(rows)
    return nchunk, assign


def _numpy_fallback(x, latent_vectors, idx):
    out = np.empty((B, D), dtype=np.float32)
    for t in np.unique(idx):
        rows = np.nonzero(idx == t)[0]
        tab = latent_vectors[t]
        invn = 1.0 / np.sqrt(np.maximum((tab * tab).sum(-1), EPS))
        sims = (x[rows] @ tab.T) * invn[None, :]
        best = np.argmax(sims, axis=-1)
        out[rows] = tab[best]
    return out


def kernel(inputs, latent_vectors, index, _trace=False, _tmpdir=None):
    from concourse import bass_utils

    x = np.asarray(inputs, dtype=np.float32).reshape(B, D)
    lv = np.ascontiguousarray(np.asarray(latent_vectors, dtype=np.float32))
    idx = np.asarray(index).astype(np.int64)

    if idx.min() < 0 or idx.max() >= T:
        return _numpy_fallback(x, lv, idx)

    invn = 1.0 / np.sqrt(np.maximum((lv * lv).sum(-1), EPS))  # (T, V)

    fast = _plan_fast(idx)
    in_maps = []
    row_map = []  # per core: list of (out_row_start, rows)
    if fast is not None:
        nc = _get_program(FAST_SEGS, 2)
        for c in range(N_CORES):
            (ta, rows_a), (tb, rows_b) = fast[c]
            xt = np.zeros((D, 3 * PCHUNK), dtype=np.float32)
            tabtn = np.empty((D, 2 * V), dtype=np.float32)
            tabr = np.empty((2 * V, D), dtype=np.float32)
            rmap = []
            for w, (t, rows) in enumerate(((ta, rows_a), (tb, rows_b))):
                main, exc = rows[:PCHUNK], rows[PCHUNK:]
                xt[:, w * PCHUNK:w * PCHUNK + len(main)] = x[main].T
                rmap.append((w * PCHUNK, main))
                if len(exc):
                    p0 = 2 * PCHUNK + w * HALFP
                    xt[:, p0:p0 + len(exc)] = x[exc].T
                    rmap.append((p0, exc))
                tabtn[:, w * V:(w + 1) * V] = (lv[t] * invn[t][:, None]).T
                tabr[w * V:(w + 1) * V, :] = lv[t]
            vb = np.zeros((PCHUNK, 1), dtype=np.uint32)
            vb[HALFP:, 0] = V
            in_maps.append({"xt": xt, "tabtn": tabtn, "tabr": tabr,
                            "vbase": vb})
            row_map.append(rmap)
    else:
        nchunk, assign = _plan_generic(idx)
        chunk_segs = tuple(((k, 0, PCHUNK),) for k in range(nchunk))
        nc = _get_program(chunk_segs, nchunk)
        for c in range(N_CORES):
            xt = np.zeros((D, nchunk * PCHUNK), dtype=np.float32)
            tabtn = np.zeros((D, nchunk * V), dtype=np.float32)
            tabr = np.zeros((nchunk * V, D), dtype=np.float32)
            rmap = []
            for k, (t, rows) in enumerate(assign[c]):
                xt[:, k * PCHUNK:k * PCHUNK + len(rows)] = x[rows].T
                tabtn[:, k * V:(k + 1) * V] = (lv[t] * invn[t][:, None]).T
                tabr[k * V:(k + 1) * V, :] = lv[t]
                rmap.append((k * PCHUNK, rows))
            in_maps.append({"xt": xt, "tabtn": tabtn, "tabr": tabr,
                            "vbase": np.zeros((PCHUNK, 1), dtype=np.uint32)})
            row_map.append(rmap)

    kw = {}
    if _trace:
        kw.update(trace=True, tmpdir=_tmpdir)
    res = bass_utils.run_bass_kernel_spmd(nc, in_maps,
                                          list(range(N_CORES)), **kw)

    out = np.empty((B, D), dtype=np.float32)
    for c in range(N_CORES):
        dev_out = res.results[c]["out"]
        for (r0, rows) in row_map[c]:
            out[rows] = dev_out[r0:r0 + len(rows)]
    if _trace:
        return out, res
    return out


# revision 16
# speedup vs baseline: 1.1928x; 1.0667x over previous
"""VQ codebook lookup (ClusteringLayer) Trainium2 kernel.

Reference semantics:
    x   = inputs.squeeze(-1)                       # (B, D)
    cur = latent_vectors[index]                    # (B, V, D)
    sim = l2norm(cur, -1) @ l2norm(x, -1)          # (B, V) cosine sims
    best = argmax(sim, -1)                         # (B,)
    out = cur[b, best[b]]                          # (B, D) un-normalized rows

Key facts used:
  * Normalizing x is a positive per-row scale -> does not change argmax.
  * sim for row b depends only on t = index[b]; there are only T=16 tables,
    so the reference's (B, V, D) gather + normalize collapses to table-level
    matmuls.
  * bf16 matmul provably flips the argmax for this input distribution
    (min fp32 top-2 gap ~8e-5), so matmul and argmax stay fp32.
  * The DVE argmax ops (max8 / find_index8) only run in 1x mode
    (~1.07 ns/elem over V=1024), so wall time is dominated by the number
    of 128-row argmax chunks. The structure below minimizes them.

Sharding (SPMD: all 8 cores execute one instruction stream, so the chunk /
segment structure must be core-independent):
  * Each core owns two tables (window 0 and window 1): chunk0 = first 128
    rows of table0, chunk1 = first 128 rows of table1.
  * Tables with more than 128 rows put their excess (<= 64 rows) in chunk2:
    partitions 0:64 hold excess of the core's OWN table0 (rhs = window 0),
    partitions 64:128 hold excess of table1 (rhs = window 1). The host
    pairs over-full tables onto the same cores so this always works for
    the contest distribution; unused partitions get zero x columns.
  * Per chunk: fp32 matmuls into a [128, 1024] PSUM tile, per-row argmax
    via max8 + find_index8, indirect-DMA gather of the winning raw rows
    (constant window offset per partition range), write out.
Host scatters rows back into batch order. A generic fallback program
(ceil(slots/8) single-table chunks per core) handles any distribution the
fast structure cannot.
"""

import os
import sys

for _p in ("/opt/trn_rl_repo", "/root/.axon_site/_ro/trn_rl_repo"):
    if os.path.isdir(_p) and _p not in sys.path:
        sys.path.insert(0, _p)

import numpy as np

# Problem constants (hardcoded per contest contract).
T, V, D = 16, 1024, 128
B = 2048
N_CORES = 8
PCHUNK = 128        # rows per chunk (PSUM partition width)
HALFP = 64          # chunk2 segment width (partition split)
NHALF = 512         # matmul free-dim half (PSUM bank limit for fp32)
WARMUP = 10          # PE p-state warm-up matmuls
EPS = 1e-12

_PROGRAM_CACHE = {}


def _emit_chunk(nc, bass, sb, ps_sim, tabn_sb, xt_sb, vb_sb, tabr, out,
                k, segs):
    """Emit one 128-row chunk: matmuls per segment, argmax, gather, out.

    segs: list of (window, p0, p1) partition segments of this chunk.
    Multi-segment chunks gather with per-row u32 base offsets (vb_sb),
    since partition-offset indirect DMAs crash the hardware DGE.
    """
    from concourse import mybir
    f32 = mybir.dt.float32
    u32 = mybir.dt.uint32

    sim_ps = ps_sim.tile([PCHUNK, V], f32, tag="sim")
    for (w, p0, p1) in segs:
        lhs = xt_sb[:, k * PCHUNK + p0:k * PCHUNK + p1]
        for h in range(V // NHALF):
            rhs = tabn_sb[w][h][:]
            nc.tensor.matmul(out=sim_ps[p0:p1, h * NHALF:(h + 1) * NHALF],
                             lhsT=lhs, rhs=rhs, start=True, stop=True)
    m8 = sb.tile([PCHUNK, 8], f32, tag=f"m8_{k}")
    nc.vector.max(out=m8[:], in_=sim_ps[:])
    v8 = sb.tile([PCHUNK, 8], u32, tag=f"v8_{k}")
    nc.vector.max_index(out=v8[:], in_max=m8[:], in_values=sim_ps[:])
    sel = sb.tile([PCHUNK, D], f32, tag=f"sel_{k}")
    if len(segs) == 1:
        nc.gpsimd.indirect_dma_start(
            out=sel[:], out_offset=None,
            in_=tabr[:],
            in_offset=bass.IndirectOffsetOnAxis(ap=v8[:, 0:1], axis=0),
            element_offset=segs[0][0] * V * D)
    else:
        goff = sb.tile([PCHUNK, 1], u32, tag=f"goff_{k}")
        nc.gpsimd.tensor_tensor(out=goff[:], in0=v8[:, 0:1],
                                in1=vb_sb[:], op=mybir.AluOpType.add)
        nc.gpsimd.indirect_dma_start(
            out=sel[:], out_offset=None,
            in_=tabr[:],
            in_offset=bass.IndirectOffsetOnAxis(ap=goff[:], axis=0))
    out_eng = nc.sync if k % 2 == 0 else nc.scalar
    out_eng.dma_start(out=out[k * PCHUNK:(k + 1) * PCHUNK, :], in_=sel[:])


def _build_program(chunk_segs, ntab):
    """Build the per-core Bass program.

    chunk_segs: per chunk, tuple of (window, p0, p1) segments.
    ntab: number of table windows.
    """
    from concourse import bacc, bass, mybir
    from concourse.tile import TileContext

    f32 = mybir.dt.float32
    nchunk = len(chunk_segs)

    nc = bacc.Bacc(None, target_bir_lowering=False, debug=False,
                   num_devices=N_CORES)
    u32 = mybir.dt.uint32
    xt = nc.declare_dram_parameter("xt", [D, nchunk * PCHUNK], f32,
                                   isOutput=False)
    tabtn = nc.declare_dram_parameter("tabtn", [D, ntab * V], f32,
                                      isOutput=False)
    tabr = nc.declare_dram_parameter("tabr", [ntab * V, D], f32,
                                     isOutput=False)
    vbase = nc.declare_dram_parameter("vbase", [PCHUNK, 1], u32,
                                      isOutput=False)
    out = nc.declare_dram_parameter("out", [nchunk * PCHUNK, D], f32,
                                    isOutput=True)

    with TileContext(nc) as tc:
        with tc.tile_pool(name="sb", bufs=1) as sb, \
             tc.tile_pool(name="ps_sim", bufs=min(nchunk, 3),
                          space="PSUM") as ps_sim, \
             tc.tile_pool(name="ps_warm", bufs=1, space="PSUM") as ps_warm:
            # ---- loads: operands of the first matmuls lead the queues.
            # One tile per (window, half) so each matmul waits only on its
            # own 256 KB slice, not the whole table load. ----
            tabn_sb = [[sb.tile([D, NHALF], f32, name=f"tn{w}_{h}")
                        for h in range(V // NHALF)] for w in range(ntab)]
            xt_sb = sb.tile([D, nchunk * PCHUNK], f32)
            vb_sb = sb.tile([PCHUNK, 1], u32)
            nc.sync.dma_start(out=xt_sb[:], in_=xt[:])
            # Three parallel HWDGE queues; window 0 (first consumed) leads.
            engs = [nc.scalar, nc.gpsimd, nc.sync]
            qi = 0
            for w in range(ntab):
                for h in range(V // NHALF):
                    c0 = w * V + h * NHALF
                    engs[qi % 3].dma_start(out=tabn_sb[w][h][:],
                                           in_=tabtn[:, c0:c0 + NHALF])
                    qi += 1
            nc.sync.dma_start(out=vb_sb[:], in_=vbase[:])

            # ---- PE warm-up during the load wait (p-state ramp) ----
            bf16 = mybir.dt.bfloat16
            ones_col_bf = nc.const_aps.tensor(1.0, (D, 1), bf16)
            ones_warm_bf = nc.const_aps.tensor(1.0, (D, NHALF), bf16)
            warm_ps = ps_warm.tile([1, NHALF], f32, tag="warm")
            for _ in range(WARMUP):
                nc.tensor.matmul(out=warm_ps[:], lhsT=ones_col_bf,
                                 rhs=ones_warm_bf, start=True, stop=True)

            for k, segs in enumerate(chunk_segs):
                _emit_chunk(nc, bass, sb, ps_sim, tabn_sb, xt_sb, vb_sb,
                            tabr, out, k, segs)
    nc.compile()
    return nc


def _get_program(chunk_segs, ntab):
    key = (chunk_segs, ntab)
    if key not in _PROGRAM_CACHE:
        _PROGRAM_CACHE[key] = _build_program(chunk_segs, ntab)
    return _PROGRAM_CACHE[key]


FAST_SEGS = (
    ((0, 0, PCHUNK),),
    ((1, 0, PCHUNK),),
    ((0, 0, HALFP), (1, HALFP, PCHUNK)),
)


def _plan_fast(idx):
    """Pair tables onto cores: chunk0/chunk1 = main 128 rows of the core's
    two tables, chunk2 = their excess rows (<=64 each). Returns None if
    infeasible (some table has more than 192 rows)."""
    tabs = []
    for t in range(T):
        rows = np.nonzero(idx == t)[0]
        if len(rows) > PCHUNK + HALFP:
            return None
        tabs.append((t, rows))
    if len(tabs) != 2 * N_CORES:
        return None
    # Sort by count descending; pair i-th with (2N-1-i)-th so over-full
    # tables spread across cores (each core gets at most 2 excesses, one
    # per window -- any pairing satisfies that since each table has one
    # excess slot and each core has one seg per window).
    tabs.sort(key=lambda s: -len(s[1]))
    assign = []
    for c in range(N_CORES):
        a = tabs[c]
        b = tabs[2 * N_CORES - 1 - c]
        assign.append((a, b))
    return assign


def _plan_generic(idx):
    """Any-distribution plan: single-table slots of <=128 rows packed onto
    cores, nchunk = ceil(slots / 8), chunk k reads window k."""
    slots = []
    for t in np.unique(idx):
        rows = np.nonzero(idx == t)[0]
        for s0 in range(0, len(rows), PCHUNK):
            slots.append((int(t), rows[s0:s0 + PCHUNK]))
    nchunk = max(1, -(-len(slots) // N_CORES))
    slots.sort(key=lambda s: -len(s[1]))
    loads = [0] * N_CORES
    assign = [[] for _ in range(N_CORES)]
    for t, rows in slots:
        c = min(range(N_CORES),
                key=lambda c: (len(assign[c]) >= nchunk, loads[c]))
        assign[c].append((t, rows))
        loads[c] += len(rows)
    return nchunk, assign


def _numpy_fallback(x, latent_vectors, idx):
    out = np.empty((B, D), dtype=np.float32)
    for t in np.unique(idx):
        rows = np.nonzero(idx == t)[0]
        tab = latent_vectors[t]
        invn = 1.0 / np.sqrt(np.maximum((tab * tab).sum(-1), EPS))
        sims = (x[rows] @ tab.T) * invn[None, :]
        best = np.argmax(sims, axis=-1)
        out[rows] = tab[best]
    return out


def kernel(inputs, latent_vectors, index, _trace=False, _tmpdir=None):
    from concourse import bass_utils

    x = np.asarray(inputs, dtype=np.float32).reshape(B, D)
    lv = np.ascontiguousarray(np.asarray(latent_vectors, dtype=np.float32))
    idx = np.asarray(index).astype(np.int64)

    if idx.min() < 0 or idx.max() >= T:
        return _numpy_fallback(x, lv, idx)

    invn = 1.0 / np.sqrt(np.maximum((lv * lv).sum(-1), EPS))  # (T, V)

    fast = _plan_fast(idx)
    in_maps = []
    row_map = []  # per core: list of (out_row_start, rows)
    if fast is not None:
        nc = _get_program(FAST_SEGS, 2)
        for c in range(N_CORES):
            (ta, rows_a), (tb, rows_b) = fast[c]
            xt = np.zeros((D, 3 * PCHUNK), dtype=np.float32)
            tabtn = np.empty((D, 2 * V), dtype=np.float32)
            tabr = np.empty((2 * V, D), dtype=np.float32)
            rmap = []
            for w, (t, rows) in enumerate(((ta, rows_a), (tb, rows_b))):
                main, exc = rows[:PCHUNK], rows[PCHUNK:]
                xt[:, w * PCHUNK:w * PCHUNK + len(main)] = x[main].T
                rmap.append((w * PCHUNK, main))
                if len(exc):
                    p0 = 2 * PCHUNK + w * HALFP
                    xt[:, p0:p0 + len(exc)] = x[exc].T
                    rmap.append((p0, exc))
                tabtn[:, w * V:(w + 1) * V] = (lv[t] * invn[t][:, None]).T
                tabr[w * V:(w + 1) * V, :] = lv[t]
            vb = np.zeros((PCHUNK, 1), dtype=np.uint32)
            vb[HALFP:, 0] = V
            in_maps.append({"xt": xt, "tabtn": tabtn, "tabr": tabr,
                            "vbase": vb})
            row_map.append(rmap)
    else:
        nchunk, assign = _plan_generic(idx)
        chunk_segs = tuple(((k, 0, PCHUNK),) for k in range(nchunk))
        nc = _get_program(chunk_segs, nchunk)
        for c in range(N_CORES):
            xt = np.zeros((D, nchunk * PCHUNK), dtype=np.float32)
            tabtn = np.zeros((D, nchunk * V), dtype=np.float32)
            tabr = np.zeros((nchunk * V, D), dtype=np.float32)
            rmap = []
            for k, (t, rows) in enumerate(assign[c]):
                xt[:, k * PCHUNK:k * PCHUNK + len(rows)] = x[rows].T
                tabtn[:, k * V:(k + 1) * V] = (lv[t] * invn[t][:, None]).T
                tabr[k * V:(k + 1) * V, :] = lv[t]
                rmap.append((k * PCHUNK, rows))
            in_maps.append({"xt": xt, "tabtn": tabtn, "tabr": tabr,
                            "vbase": np.zeros((PCHUNK, 1), dtype=np.uint32)})
            row_map.append(rmap)

    kw = {}
    if _trace:
        kw.update(trace=True, tmpdir=_tmpdir)
    res = bass_utils.run_bass_kernel_spmd(nc, in_maps,
                                          list(range(N_CORES)), **kw)

    out = np.empty((B, D), dtype=np.float32)
    for c in range(N_CORES):
        dev_out = res.results[c]["out"]
        for (r0, rows) in row_map[c]:
            out[rows] = dev_out[r0:r0 + len(rows)]
    if _trace:
        return out, res
    return out
